# revision 14
# baseline (speedup 1.0000x reference)
"""Bistable Recurrent Cell layer on 8 Trainium2 NeuronCores.

Strategy (data-parallel over batch):
  - B=128 sharded over 8 cores (16 rows each); weights replicated.
  - Per core, everything runs in a "transposed" layout with the hidden dim on
    SBUF partitions: tiles are [128 partitions = h%128, (c,b) free] where
    c = h//128 (4 chunks) and b = local batch (16).
  - Input projections xr/xz/xh are computed on the tensor engine in bf16 from
    a host-pre-transposed copy of x (xT[d, t, b]), in T-blocks, accumulating
    over 2 K-chunks of D=256, then copied PSUM->SBUF on the scalar engine.
  - The recurrence runs 512 sequential steps of vector/scalar ops on
    [128, 64] fp32 tiles:
        r  = 1 + tanh(pr + h*mr)  = 2*sigmoid(2*pr + 2*h*mr)
        z  = sigmoid(pz + h*mz)
        u  = tanh(ph + r*h)       = tanh(2*(ph/2 + sigmoid_r*h))
        h' = u + z*(h - u)
    The x2 / x0.5 factors are folded into the GEMM weights so that both
    sigmoids become one fused activation over a [128,128] concat tile.
  - Output h_t is staged per T-block in SBUF and DMA'd to a transposed DRAM
    buffer yT[p, (t,c,b)]; the host undoes the transpose.
"""

import os
import sys

import numpy as np

for _p in ("/opt/trn_rl_repo",):
    if _p not in sys.path and os.path.isdir(_p):
        sys.path.insert(0, _p)

import concourse.bass as bass
import concourse.bacc as bacc
import concourse.mybir as mybir
from concourse import bass_utils
from concourse.tile import TileContext

try:
    from ml_dtypes import bfloat16 as _bf16_np
except ImportError:  # pragma: no cover
    import jax.numpy as _jnp

    _bf16_np = _jnp.bfloat16

F32 = mybir.dt.float32
BF16 = mybir.dt.bfloat16
ALU = mybir.AluOpType
AF = mybir.ActivationFunctionType

B, T, D, H = 128, 512, 256, 512
NCORES = 8
BL = B // NCORES          # local batch = 16
C = H // 128              # h chunks = 4
COLS = C * BL             # free width of a state tile = 64
KCH = D // 128            # contraction chunks = 2


def build_program(t_total=T, tblk=128, gates_ones=True, biases_zero=True,
                  gemm_dt=BF16):
    """Emit the per-core Bass program. Returns nc."""
    nb = t_total // tblk
    ncols_blk = tblk * BL          # gemm moving cols per block per k-chunk
    nsub = max(1, ncols_blk // 512)  # 512-col sub-blocks (PSUM bank size)
    sub_cols = ncols_blk // nsub

    nc = bacc.Bacc("TRN2", target_bir_lowering=False, debug=False)

    xT = nc.dram_tensor("xT", [D, t_total * BL], gemm_dt, kind="ExternalInput").ap()
    h0T = nc.dram_tensor("h0T", [128, COLS], F32, kind="ExternalInput").ap()
    # weights: wr = kr, wz05 = 0.5*kz, wh2 = 0.5*kh (one sigmoid op at scale=2)
    w_dram = [
        nc.dram_tensor(n, [D, H], gemm_dt, kind="ExternalInput").ap()
        for n in ("wr", "wz", "wh2")
    ]
    # general-path tensors (tiny; always declared, conditionally used)
    mrt = nc.dram_tensor("mrt", [128, COLS], F32, kind="ExternalInput").ap()
    mzt = nc.dram_tensor("mzt", [128, COLS], F32, kind="ExternalInput").ap()
    biasrow = nc.dram_tensor("biasrow", [1, 2 * H], gemm_dt, kind="ExternalInput").ap()
    yT = nc.dram_tensor("yT", [128, t_total * COLS], F32, kind="ExternalOutput").ap()

    with TileContext(nc) as tc:
        with (
            tc.tile_pool(name="const", bufs=1) as cpool,
            tc.tile_pool(name="xk", bufs=2) as xpool,
            tc.tile_pool(name="proj", bufs=2) as ppool,
            tc.tile_pool(name="outb", bufs=2) as opool,
            tc.tile_pool(name="step", bufs=8) as spool,
            tc.tile_pool(name="psum", bufs=6, space="PSUM") as psp,
        ):
            # ---- constants / weights ----
            w_sb = []  # w_sb[p][k] : [128, H] bf16
            for p in range(3):
                per_k = []
                for k in range(KCH):
                    wt = cpool.tile([128, H], gemm_dt, tag=f"w{p}{k}")
                    nc.sync.dma_start(out=wt, in_=w_dram[p][k * 128:(k + 1) * 128, :])
                    per_k.append(wt)
                w_sb.append(per_k)

            hprev = cpool.tile([128, COLS], F32, tag="hprev")
            nc.sync.dma_start(out=hprev, in_=h0T)

            if not gates_ones:
                mr_sb = cpool.tile([128, COLS], F32, tag="mr")
                mz_sb = cpool.tile([128, COLS], F32, tag="mz")
                nc.sync.dma_start(out=mr_sb, in_=mrt)
                nc.sync.dma_start(out=mz_sb, in_=mzt)
            if not biases_zero:
                ones_sb = cpool.tile([1, 512], gemm_dt, tag="ones")
                nc.vector.memset(ones_sb, 1.0)
                brow_sb = cpool.tile([1, 2 * H], BF16, tag="brow")
                nc.sync.dma_start(out=brow_sb, in_=biasrow)

            out_tiles = []
            for blk in range(nb):
                # ---- load x block (both k-chunks) ----
                xk = []
                for k in range(KCH):
                    xt = xpool.tile([128, ncols_blk], gemm_dt, tag=f"x{k}")
                    # split DMA over sub-chunks for queue parallelism
                    for s in range(nsub):
                        nc.sync.dma_start(
                            out=xt[:, s * sub_cols:(s + 1) * sub_cols],
                            in_=xT[k * 128:(k + 1) * 128,
                                   blk * ncols_blk + s * sub_cols:
                                   blk * ncols_blk + (s + 1) * sub_cols],
                        )
                    xk.append(xt)

                # ---- projections: P[p] cols = (c, t, b) ----
                P = []
                for p in range(3):
                    Pt = ppool.tile([128, C * ncols_blk], F32, tag=f"P{p}")
                    P.append(Pt)
                for p in (1, 0, 2):
                    for c in range(C):
                        psums = []
                        for s in range(nsub):
                            ps = psp.tile([128, sub_cols], F32, tag="mm")
                            psums.append(ps)
                        for k in range(KCH):
                            for s in range(nsub):
                                nc.tensor.matmul(
                                    psums[s],
                                    w_sb[p][k][:, c * 128:(c + 1) * 128],
                                    xk[k][:, s * sub_cols:(s + 1) * sub_cols],
                                    start=(k == 0),
                                    stop=(k == KCH - 1 and (biases_zero or p == 2)),
                                )
                        if not biases_zero and p < 2:
                            # += bias via K=1 matmul with a ones row
                            for s in range(nsub):
                                nc.tensor.matmul(
                                    psums[s],
                                    brow_sb[:, p * H + c * 128:
                                            p * H + (c + 1) * 128],
                                    ones_sb[:, :sub_cols],
                                    start=False,
                                    stop=True,
                                )
                        for s in range(nsub):
                            nc.scalar.copy(
                                P[p][:, c * ncols_blk + s * sub_cols:
                                     c * ncols_blk + (s + 1) * sub_cols],
                                psums[s],
                            )

                # per-step views: [128, t, (c,b)]
                Pv = [P[p].rearrange("P (c t b) -> P t c b", c=C, t=tblk, b=BL)
                      for p in range(3)]

                OUT = opool.tile([128, tblk * COLS], F32, tag="OUT")
                out_tiles.append(OUT)

                # ---- recurrence ----
                for t in range(tblk):
                    if blk == 0 and t == 0:
                        h = hprev
                    elif t == 0:
                        prev = out_tiles[blk - 1]
                        h = prev[:, (tblk - 1) * COLS: tblk * COLS]
                    else:
                        h = OUT[:, (t - 1) * COLS: t * COLS]

                    # ab = [pr + h*mr | 0.5*pz + 0.5*h*mz]; one sigmoid at
                    # scale=2 gives [sr | z] with r = 2*sr.
                    ab = spool.tile([128, 2 * COLS], F32, tag="ab")
                    if gates_ones:
                        nc.vector.tensor_add(ab[:, :COLS], h, Pv[0][:, t])
                        nc.vector.tensor_add(ab[:, COLS:], h, Pv[1][:, t])
                    else:
                        tmp = spool.tile([128, COLS], F32, tag="gtmp")
                        nc.vector.tensor_mul(tmp, h, mr_sb)
                        nc.vector.tensor_add(ab[:, :COLS], tmp, Pv[0][:, t])
                        tmp2 = spool.tile([128, COLS], F32, tag="gtmp2")
                        nc.vector.tensor_mul(tmp2, h, mz_sb)
                        nc.vector.tensor_add(ab[:, COLS:], tmp2, Pv[1][:, t])

                    ss = spool.tile([128, 2 * COLS], F32, tag="ss")
                    nc.scalar.activation(ss[:, :COLS], ab[:, :COLS],
                                         AF.Sigmoid, scale=2.0)
                    nc.scalar.activation(ss[:, COLS:], ab[:, COLS:], AF.Sigmoid)

                    m = spool.tile([128, COLS], F32, tag="m")
                    nc.vector.tensor_mul(m, ss[:, :COLS], h)
                    ua = spool.tile([128, COLS], F32, tag="ua")
                    nc.vector.tensor_add(ua, m, Pv[2][:, t])
                    u = spool.tile([128, COLS], F32, tag="u")
                    nc.scalar.activation(u, ua, AF.Tanh, scale=2.0)

                    e = spool.tile([128, COLS], F32, tag="e")
                    nc.vector.tensor_sub(e, h, u)
                    f = spool.tile([128, COLS], F32, tag="f")
                    nc.vector.tensor_mul(f, ss[:, COLS:], e)
                    nc.vector.tensor_add(OUT[:, t * COLS:(t + 1) * COLS], f, u)

                # ---- store block ----
                st_chunks = 4
                st_w = tblk * COLS // st_chunks
                for s in range(st_chunks):
                    nc.sync.dma_start(
                        out=yT[:, blk * tblk * COLS + s * st_w:
                               blk * tblk * COLS + (s + 1) * st_w],
                        in_=OUT[:, s * st_w:(s + 1) * st_w],
                    )
    nc.finalize()
    return nc


def _to_tiles(v):
    """[H] host vector -> [128, COLS] tile layout t[p, c*BL+b] = v[c*128+p]."""
    m = np.empty((128, COLS), np.float32)
    for c in range(C):
        m[:, c * BL:(c + 1) * BL] = v[c * 128:(c + 1) * 128, None]
    return m


def kernel(x, h0, kernelr, kernelz, kernelh, memoryr, memoryz, br, bz,
           _t_total=T, _tblk=64, _trace=False, _gemm="bf16"):
    x = np.asarray(x, np.float32)
    h0 = np.asarray(h0, np.float32)
    kernelr = np.asarray(kernelr, np.float32)
    kernelz = np.asarray(kernelz, np.float32)
    kernelh = np.asarray(kernelh, np.float32)
    memoryr = np.asarray(memoryr, np.float32)
    memoryz = np.asarray(memoryz, np.float32)
    br = np.asarray(br, np.float32)
    bz = np.asarray(bz, np.float32)

    t_total = _t_total
    gates_ones = bool(np.all(memoryr == 1.0) and np.all(memoryz == 1.0))
    biases_zero = bool(np.all(br == 0.0) and np.all(bz == 0.0))

    gdt = {"bf16": BF16, "f32": F32, "f32r": mybir.dt.float32r}[_gemm]
    gnp = _bf16_np if _gemm == "bf16" else np.float32
    nc = build_program(t_total=t_total, tblk=_tblk,
                       gates_ones=gates_ones, biases_zero=biases_zero,
                       gemm_dt=gdt)

    # host-side weight prep (shared across cores)
    wr = kernelr.astype(gnp)
    wz = kernelz.astype(gnp)
    wh2 = (0.5 * kernelh).astype(gnp)
    mrt = _to_tiles(memoryr)
    mzt = _to_tiles(memoryz)
    biasrow = np.concatenate([br, bz]).astype(gnp)[None, :]

    in_maps = []
    for i in range(NCORES):
        bs, be = i * BL, (i + 1) * BL
        # xT[d, t*BL+b] = x[bs+b, t, d]
        xTi = np.ascontiguousarray(
            x[bs:be, :t_total].transpose(2, 1, 0).reshape(D, t_total * BL)
        ).astype(gnp)
        # h0T[p, c*BL+b] = h0[bs+b, c*128+p]
        h0Ti = np.ascontiguousarray(
            h0[bs:be].reshape(BL, C, 128).transpose(2, 1, 0).reshape(128, COLS))
        in_maps.append({
            "xT": xTi, "h0T": h0Ti,
            "wr": wr, "wz": wz, "wh2": wh2,
            "mrt": mrt, "mzt": mzt, "biasrow": biasrow,
        })

    res = bass_utils.run_bass_kernel_spmd(
        nc, in_maps, core_ids=list(range(NCORES)), trace=_trace)

    y = np.empty((B, t_total, H), np.float32)
    for i in range(NCORES):
        yTi = res.results[i]["yT"]  # [128, t*COLS]
        yi = yTi.reshape(128, t_total, C, BL).transpose(3, 1, 2, 0)
        y[i * BL:(i + 1) * BL] = yi.reshape(BL, t_total, H)
    if _trace:
        kernel._last_exec_time_ns = res.exec_time_ns
    return y


# revision 16
# speedup vs baseline: 1.2291x; 1.2291x over previous
"""Bistable Recurrent Cell layer on 8 Trainium2 NeuronCores.

Strategy (data-parallel over batch):
  - B=128 sharded over 8 cores (16 rows each); weights replicated.
  - Per core, everything runs in a "transposed" layout with the hidden dim on
    SBUF partitions: tiles are [128 partitions = h%128, (c,b) free] where
    c = h//128 (4 chunks) and b = local batch (16).
  - Input projections xr/xz/xh are computed on the tensor engine in bf16 from
    a host-pre-transposed copy of x (xT[d, t, b]), in T-blocks, accumulating
    over 2 K-chunks of D=256, then copied PSUM->SBUF on the scalar engine.
  - The recurrence runs 512 sequential steps of vector/scalar ops on
    [128, 64] fp32 tiles:
        r  = 1 + tanh(pr + h*mr)  = 2*sigmoid(2*pr + 2*h*mr)
        z  = sigmoid(pz + h*mz)
        u  = tanh(ph + r*h)       = tanh(2*(ph/2 + sigmoid_r*h))
        h' = u + z*(h - u)
    The x2 / x0.5 factors are folded into the GEMM weights so that both
    sigmoids become one fused activation over a [128,128] concat tile.
  - Output h_t is staged per T-block in SBUF and DMA'd to a transposed DRAM
    buffer yT[p, (t,c,b)]; the host undoes the transpose.
"""

import os
import sys

import numpy as np

for _p in ("/opt/trn_rl_repo",):
    if _p not in sys.path and os.path.isdir(_p):
        sys.path.insert(0, _p)

import concourse.bass as bass
import concourse.bacc as bacc
import concourse.mybir as mybir
from concourse import bass_utils
from concourse.tile import TileContext

try:
    from ml_dtypes import bfloat16 as _bf16_np
except ImportError:  # pragma: no cover
    import jax.numpy as _jnp

    _bf16_np = _jnp.bfloat16

F32 = mybir.dt.float32
BF16 = mybir.dt.bfloat16
ALU = mybir.AluOpType
AF = mybir.ActivationFunctionType

B, T, D, H = 128, 512, 256, 512
NCORES = 8
BL = B // NCORES          # local batch = 16
C = H // 128              # h chunks = 4
COLS = C * BL             # free width of a state tile = 64
KCH = D // 128            # contraction chunks = 2


_DESYNC_TYPES = ("InstTensorTensor", "InstTensorScalarPtr", "InstActivation",
                 "InstMemset")


def _desync_same_engine(nc):
    """Demote same-engine compute->compute sync deps to nosync ordering.

    DVE/ACT execute their instruction streams in order (the per-op DRAIN is
    the output-dependency barrier), so a semaphore between two ops on the
    same engine only adds ~90ns of wait-processing per op. Keep the
    dependency for the scheduler, drop the semaphore.
    """
    imap = nc.inst_map
    for inst in list(imap.values()):
        if type(inst).__name__ not in _DESYNC_TYPES:
            continue
        eng = getattr(inst, "engine", None)
        if eng not in (mybir.EngineType.DVE, mybir.EngineType.Activation):
            continue
        syncs = list(inst.sync_dependency_names())
        keep, demote = [], []
        for d in syncs:
            di = imap.get(d)
            if (di is not None and type(di).__name__ in _DESYNC_TYPES
                    and getattr(di, "engine", None) == eng):
                demote.append(d)
            else:
                keep.append(d)
        if demote:
            sset = inst.sync_dependency_set_copy()
            nset = inst.nosync_dependency_set_copy()
            for d in demote:
                sset.discard(d)
                nset.add(d)
            inst.set_sync_dependencies(sset)
            inst.set_nosync_dependencies(nset)


def build_program(t_total=T, tblk=128, gates_ones=True, biases_zero=True,
                  gemm_dt=BF16, desync=True):
    """Emit the per-core Bass program. Returns nc."""
    nb = t_total // tblk
    ncols_blk = tblk * BL          # gemm moving cols per block per k-chunk
    nsub = max(1, ncols_blk // 512)  # 512-col sub-blocks (PSUM bank size)
    sub_cols = ncols_blk // nsub

    nc = bacc.Bacc("TRN2", target_bir_lowering=False, debug=False)

    xT = nc.dram_tensor("xT", [D, t_total * BL], gemm_dt, kind="ExternalInput").ap()
    h0T = nc.dram_tensor("h0T", [128, COLS], F32, kind="ExternalInput").ap()
    # weights: wr = kr, wz05 = 0.5*kz, wh2 = 0.5*kh (one sigmoid op at scale=2)
    w_dram = [
        nc.dram_tensor(n, [D, H], gemm_dt, kind="ExternalInput").ap()
        for n in ("wr", "wz", "wh2")
    ]
    # general-path tensors (tiny; always declared, conditionally used)
    mrt = nc.dram_tensor("mrt", [128, COLS], F32, kind="ExternalInput").ap()
    mzt = nc.dram_tensor("mzt", [128, COLS], F32, kind="ExternalInput").ap()
    biasrow = nc.dram_tensor("biasrow", [1, 2 * H], gemm_dt, kind="ExternalInput").ap()
    yT = nc.dram_tensor("yT", [128, t_total * COLS], F32, kind="ExternalOutput").ap()

    with TileContext(nc) as tc:
        with (
            tc.tile_pool(name="const", bufs=1) as cpool,
            tc.tile_pool(name="xk", bufs=2) as xpool,
            tc.tile_pool(name="proj", bufs=2) as ppool,
            tc.tile_pool(name="outb", bufs=2) as opool,
            tc.tile_pool(name="step", bufs=8) as spool,
            tc.tile_pool(name="psum", bufs=6, space="PSUM") as psp,
        ):
            # ---- constants / weights ----
            w_sb = []  # w_sb[p][k] : [128, H] bf16
            for p in range(3):
                per_k = []
                for k in range(KCH):
                    wt = cpool.tile([128, H], gemm_dt, tag=f"w{p}{k}")
                    nc.sync.dma_start(out=wt, in_=w_dram[p][k * 128:(k + 1) * 128, :])
                    per_k.append(wt)
                w_sb.append(per_k)

            hprev = cpool.tile([128, COLS], F32, tag="hprev")
            nc.sync.dma_start(out=hprev, in_=h0T)

            if not gates_ones:
                mr_sb = cpool.tile([128, COLS], F32, tag="mr")
                mz_sb = cpool.tile([128, COLS], F32, tag="mz")
                nc.sync.dma_start(out=mr_sb, in_=mrt)
                nc.sync.dma_start(out=mz_sb, in_=mzt)
            if not biases_zero:
                ones_sb = cpool.tile([1, 512], gemm_dt, tag="ones")
                nc.vector.memset(ones_sb, 1.0)
                brow_sb = cpool.tile([1, 2 * H], BF16, tag="brow")
                nc.sync.dma_start(out=brow_sb, in_=biasrow)

            out_tiles = []
            for blk in range(nb):
                # ---- load x block (both k-chunks) ----
                xk = []
                for k in range(KCH):
                    xt = xpool.tile([128, ncols_blk], gemm_dt, tag=f"x{k}")
                    # split DMA over sub-chunks for queue parallelism
                    for s in range(nsub):
                        nc.sync.dma_start(
                            out=xt[:, s * sub_cols:(s + 1) * sub_cols],
                            in_=xT[k * 128:(k + 1) * 128,
                                   blk * ncols_blk + s * sub_cols:
                                   blk * ncols_blk + (s + 1) * sub_cols],
                        )
                    xk.append(xt)

                # ---- projections: P[p] cols = (c, t, b) ----
                P = []
                for p in range(3):
                    Pt = ppool.tile([128, C * ncols_blk], F32, tag=f"P{p}")
                    P.append(Pt)
                for p in (1, 0, 2):
                    for c in range(C):
                        psums = []
                        for s in range(nsub):
                            ps = psp.tile([128, sub_cols], F32, tag="mm")
                            psums.append(ps)
                        for k in range(KCH):
                            for s in range(nsub):
                                nc.tensor.matmul(
                                    psums[s],
                                    w_sb[p][k][:, c * 128:(c + 1) * 128],
                                    xk[k][:, s * sub_cols:(s + 1) * sub_cols],
                                    start=(k == 0),
                                    stop=(k == KCH - 1 and (biases_zero or p == 2)),
                                )
                        if not biases_zero and p < 2:
                            # += bias via K=1 matmul with a ones row
                            for s in range(nsub):
                                nc.tensor.matmul(
                                    psums[s],
                                    brow_sb[:, p * H + c * 128:
                                            p * H + (c + 1) * 128],
                                    ones_sb[:, :sub_cols],
                                    start=False,
                                    stop=True,
                                )
                        for s in range(nsub):
                            nc.scalar.copy(
                                P[p][:, c * ncols_blk + s * sub_cols:
                                     c * ncols_blk + (s + 1) * sub_cols],
                                psums[s],
                            )

                # per-step views: [128, t, (c,b)]
                Pv = [P[p].rearrange("P (c t b) -> P t c b", c=C, t=tblk, b=BL)
                      for p in range(3)]

                OUT = opool.tile([128, tblk * COLS], F32, tag="OUT")
                out_tiles.append(OUT)

                # ---- recurrence ----
                for t in range(tblk):
                    if blk == 0 and t == 0:
                        h = hprev
                    elif t == 0:
                        prev = out_tiles[blk - 1]
                        h = prev[:, (tblk - 1) * COLS: tblk * COLS]
                    else:
                        h = OUT[:, (t - 1) * COLS: t * COLS]

                    # ab = [pr + h*mr | 0.5*pz + 0.5*h*mz]; one sigmoid at
                    # scale=2 gives [sr | z] with r = 2*sr.
                    ab = spool.tile([128, 2 * COLS], F32, tag="ab")
                    if gates_ones:
                        nc.vector.tensor_add(ab[:, :COLS], h, Pv[0][:, t])
                        nc.vector.tensor_add(ab[:, COLS:], h, Pv[1][:, t])
                    else:
                        tmp = spool.tile([128, COLS], F32, tag="gtmp")
                        nc.vector.tensor_mul(tmp, h, mr_sb)
                        nc.vector.tensor_add(ab[:, :COLS], tmp, Pv[0][:, t])
                        tmp2 = spool.tile([128, COLS], F32, tag="gtmp2")
                        nc.vector.tensor_mul(tmp2, h, mz_sb)
                        nc.vector.tensor_add(ab[:, COLS:], tmp2, Pv[1][:, t])

                    ss = spool.tile([128, 2 * COLS], F32, tag="ss")
                    nc.scalar.activation(ss[:, :COLS], ab[:, :COLS],
                                         AF.Sigmoid, scale=2.0)
                    nc.scalar.activation(ss[:, COLS:], ab[:, COLS:], AF.Sigmoid)

                    m = spool.tile([128, COLS], F32, tag="m")
                    nc.vector.tensor_mul(m, ss[:, :COLS], h)
                    ua = spool.tile([128, COLS], F32, tag="ua")
                    nc.vector.tensor_add(ua, m, Pv[2][:, t])
                    u = spool.tile([128, COLS], F32, tag="u")
                    nc.scalar.activation(u, ua, AF.Tanh, scale=2.0)

                    e = spool.tile([128, COLS], F32, tag="e")
                    nc.vector.tensor_sub(e, h, u)
                    f = spool.tile([128, COLS], F32, tag="f")
                    nc.vector.tensor_mul(f, ss[:, COLS:], e)
                    nc.vector.tensor_add(OUT[:, t * COLS:(t + 1) * COLS], f, u)

                # ---- store block ----
                st_chunks = 4
                st_w = tblk * COLS // st_chunks
                for s in range(st_chunks):
                    nc.sync.dma_start(
                        out=yT[:, blk * tblk * COLS + s * st_w:
                               blk * tblk * COLS + (s + 1) * st_w],
                        in_=OUT[:, s * st_w:(s + 1) * st_w],
                    )
            if desync:
                _desync_same_engine(nc)
    nc.finalize()
    return nc


def _to_tiles(v):
    """[H] host vector -> [128, COLS] tile layout t[p, c*BL+b] = v[c*128+p]."""
    m = np.empty((128, COLS), np.float32)
    for c in range(C):
        m[:, c * BL:(c + 1) * BL] = v[c * 128:(c + 1) * 128, None]
    return m


def kernel(x, h0, kernelr, kernelz, kernelh, memoryr, memoryz, br, bz,
           _t_total=T, _tblk=64, _trace=False, _gemm="bf16"):
    x = np.asarray(x, np.float32)
    h0 = np.asarray(h0, np.float32)
    kernelr = np.asarray(kernelr, np.float32)
    kernelz = np.asarray(kernelz, np.float32)
    kernelh = np.asarray(kernelh, np.float32)
    memoryr = np.asarray(memoryr, np.float32)
    memoryz = np.asarray(memoryz, np.float32)
    br = np.asarray(br, np.float32)
    bz = np.asarray(bz, np.float32)

    t_total = _t_total
    gates_ones = bool(np.all(memoryr == 1.0) and np.all(memoryz == 1.0))
    biases_zero = bool(np.all(br == 0.0) and np.all(bz == 0.0))

    gdt = {"bf16": BF16, "f32": F32, "f32r": mybir.dt.float32r}[_gemm]
    gnp = _bf16_np if _gemm == "bf16" else np.float32
    nc = build_program(t_total=t_total, tblk=_tblk,
                       gates_ones=gates_ones, biases_zero=biases_zero,
                       gemm_dt=gdt)

    # host-side weight prep (shared across cores)
    wr = kernelr.astype(gnp)
    wz = kernelz.astype(gnp)
    wh2 = (0.5 * kernelh).astype(gnp)
    mrt = _to_tiles(memoryr)
    mzt = _to_tiles(memoryz)
    biasrow = np.concatenate([br, bz]).astype(gnp)[None, :]

    in_maps = []
    for i in range(NCORES):
        bs, be = i * BL, (i + 1) * BL
        # xT[d, t*BL+b] = x[bs+b, t, d]
        xTi = np.ascontiguousarray(
            x[bs:be, :t_total].transpose(2, 1, 0).reshape(D, t_total * BL)
        ).astype(gnp)
        # h0T[p, c*BL+b] = h0[bs+b, c*128+p]
        h0Ti = np.ascontiguousarray(
            h0[bs:be].reshape(BL, C, 128).transpose(2, 1, 0).reshape(128, COLS))
        in_maps.append({
            "xT": xTi, "h0T": h0Ti,
            "wr": wr, "wz": wz, "wh2": wh2,
            "mrt": mrt, "mzt": mzt, "biasrow": biasrow,
        })

    res = bass_utils.run_bass_kernel_spmd(
        nc, in_maps, core_ids=list(range(NCORES)), trace=_trace)

    y = np.empty((B, t_total, H), np.float32)
    for i in range(NCORES):
        yTi = res.results[i]["yT"]  # [128, t*COLS]
        yi = yTi.reshape(128, t_total, C, BL).transpose(3, 1, 2, 0)
        y[i * BL:(i + 1) * BL] = yi.reshape(BL, t_total, H)
    if _trace:
        kernel._last_exec_time_ns = res.exec_time_ns
    return y
